# revision 11
# baseline (speedup 1.0000x reference)
"""BarycenterNorm (eval mode) Trainium2 kernel.

Math: the reference chain collapses to out_i = exp(T log(T X_i T^T) T^T)
with T = chol(B^-1).T (verified vs reference to 5e-9). With X and L
symmetric, both congruences avoid per-sample transposes on the PE
(out = lhsT.T @ rhs):
  A1 = X_i T^T    per-sample stationary X_i, shared moving T^T  (f32 quad)
  Y  = T A1       shared stationary diag(aT^T,aT^T), f32r       -> W
  W  = a*Y + b*I  spectrum in [-1,1]
  2L = cheb2-Clenshaw(W)    all-bf16, two-level Chebyshev basis:
       p(W) = sum_j Q_j(W) T_j(z), z = T_s(W); doubled recurrence
       Bt_j = 2(z Bt_{j+1} + Q_j) - Bt_{j+2} keeps the x2 out of Q.
  A2 = (2L) T^T   per-sample stationary, f32 quad
  V  = T A2/(2zb) shared f32r; spectrum [-1,1]
  out = p_exp(V)  monomial PS (coeffs O(1)), bf16

Layout: 16-sample groups; tiles [128,512]: samples 16g..16g+7 in
partitions 0-63, 16g+8..16g+15 in 64-127, each a [64,64] block along
free. Per-sample matmuls are 64x64 quadrant matmuls at (0,0)/(64,64).
Coefficient injection: K=128 (c*I) matmuls accumulate in PSUM; they must
be emitted BEFORE the quadrant matmuls of the same bank (HW quirk:
full-K accumulation after quadrant-mode start=True writes corrupts).
"""
import os
import sys

import numpy as np

sys.path.insert(0, "/opt/trn_rl_repo")

import concourse.bacc as bacc  # noqa: E402
import concourse.tile as tile  # noqa: E402
from concourse import mybir  # noqa: E402
from concourse import bass_utils  # noqa: E402

try:
    import axon_profile_shim  # noqa: F401
except Exception:
    pass

F32 = mybir.dt.float32
F32R = mybir.dt.float32r
BF16 = mybir.dt.bfloat16

C = 64
BATCH = 8192
NCORES = 8
SPC = BATCH // NCORES
NGRP = SPC // 16

N_LOG = int(os.environ.get("K_NLOG", 15))
S_LOG = int(os.environ.get("K_SLOG", 4))
N_EXP = int(os.environ.get("K_NEXP", 7))
S_EXP = int(os.environ.get("K_SEXP", 3))
A_MARGIN = 0.98
B_MARGIN = 6.5

MULT = mybir.AluOpType.mult
ADD = mybir.AluOpType.add
SUB = mybir.AluOpType.subtract
COPYF = mybir.ActivationFunctionType.Copy


def _cheb_coeffs(f, lo, hi, deg):
    k = np.arange(deg + 1)
    nw = np.cos((2 * k + 1) * np.pi / (2 * (deg + 1)))
    nx = 0.5 * (hi - lo) * nw + 0.5 * (lo + hi)
    return np.polynomial.chebyshev.chebfit(nw, f(nx), deg)


def _solve_two_level(a, s):
    """p = sum_{j,i} b[j,i] T_i(w) T_{js}(w); triangular solve."""
    n = len(a) - 1
    r = n // s
    rem = a.astype(np.float64).copy()
    b = np.zeros((r + 1, s))
    for j in range(r, -1, -1):
        for i in range(min(s - 1, n - j * s), 0, -1):
            m = j * s + i
            if j == 0:
                b[j, i] = rem[m]
                rem[m] = 0.0
            else:
                coef = 2.0 * rem[m]
                b[j, i] = coef
                rem[m] = 0.0
                rem[abs(j * s - i)] -= coef / 2.0
        b[j, 0] = rem[j * s]
        rem[j * s] = 0.0
    assert np.abs(rem).max() < 1e-10
    return b


def _host_prep(running_mean):
    B = running_mean[0].astype(np.float64)
    sev = np.linalg.eigvalsh(B)
    T = np.linalg.cholesky(np.linalg.inv(B)).T  # upper; T^T T = B^-1
    a = A_MARGIN / sev[-1]
    b = B_MARGIN / sev[0]
    alpha = 2.0 / (b - a)
    beta = -(a + b) / (b - a)
    zb = max(abs(np.log(a)), abs(np.log(b))) / sev[0]
    b_log = _solve_two_level(_cheb_coeffs(np.log, a, b, N_LOG), S_LOG)
    c_exp = np.polynomial.chebyshev.cheb2poly(
        _cheb_coeffs(lambda v: np.exp(zb * v), -1.0, 1.0, N_EXP))
    return dict(T=T, alpha=alpha, beta=beta, zb=zb, b_log=b_log,
                c_exp=c_exp, r_log=N_LOG // S_LOG, r_exp=N_EXP // S_EXP)


def _build_consts(hp):
    T = hp["T"].astype(np.float32)
    Tt = np.ascontiguousarray(T.T)
    alpha = np.float32(hp["alpha"])
    zb = np.float32(hp["zb"])
    I64 = np.eye(64, dtype=np.float32)

    slabs = {}
    cols = []

    def add(name, arr):
        c0 = sum(a.shape[1] for a in cols)
        cols.append(np.ascontiguousarray(arr, dtype=np.float32))
        slabs[name] = (c0, c0 + arr.shape[1])

    dTT = np.zeros((128, 128), np.float32)
    dTT[:64, :64] = Tt
    dTT[64:, 64:] = Tt
    add("S1", alpha * dTT)
    add("S2", dTT / (2.0 * zb))
    add("Tmov", np.concatenate([Tt, Tt], axis=0))  # [128, 64]

    ipat = np.zeros((128, 512), np.float32)
    for j in range(8):
        ipat[:64, 64 * j:64 * j + 64] = I64
        ipat[64:, 64 * j:64 * j + 64] = I64

    add("IP_beta", np.float32(hp["beta"]) * ipat)
    add("IP_unit", ipat)

    bexp_blocks = [hp["c_exp"][j * S_EXP:min(j * S_EXP + S_EXP, N_EXP + 1)]
                   for j in range(hp["r_exp"] + 1)]
    for j, blk in enumerate(bexp_blocks):
        add(f"IP_exp{j}", np.float32(blk[0]) * ipat)

    I128 = np.eye(128, dtype=np.float32)
    b_log = hp["b_log"]
    for j in range(hp["r_log"] + 1):
        for i in range(S_LOG):
            v = b_log[j, i] * (2.0 if j == 0 else 1.0)
            if v != 0.0:
                add(f"CL{j}_{i}", np.float32(v) * I128)
    for j, blk in enumerate(bexp_blocks):
        for i in range(1, len(blk)):
            add(f"CE{j}_{i}", np.float32(blk[i]) * I128)

    return np.concatenate(cols, axis=1), slabs, bexp_blocks


def _build_kernel(hp, ngrp=None):
    if ngrp is None:
        ngrp = int(os.environ.get("K_NGRP", NGRP))
    blob, slabs, bexp_blocks = _build_consts(hp)
    r_log, r_exp = hp["r_log"], hp["r_exp"]

    nc = bacc.Bacc("TRN2", target_bir_lowering=False, debug=False)
    x_d = nc.dram_tensor("x", [SPC, C, C], F32, kind="ExternalInput").ap()
    cst_d = nc.dram_tensor("cst", [128, blob.shape[1]], F32,
                           kind="ExternalInput").ap()
    out_d = nc.dram_tensor("out", [SPC, C, C], F32, kind="ExternalOutput").ap()

    x_r = x_d.rearrange("(g two p) r c -> g two r p c", g=NGRP, two=2)
    o_r = out_d.rearrange("(g two p) r c -> g two r p c", g=NGRP, two=2)

    with tile.TileContext(nc) as tc:
        with tc.tile_pool(name="csts", bufs=1) as csts, \
             tc.tile_pool(name="work", bufs=2) as work, \
             tc.tile_pool(name="psp", bufs=5, space="PSUM") as psp:

            _cst_cache = {}

            def cslab(name, dtype=F32):
                c0, c1 = slabs[name]
                if name in _cst_cache:
                    t = _cst_cache[name]
                else:
                    t = csts.tile([128, c1 - c0], F32, name=f"cst_{name}",
                                  tag=f"cst_{name}")
                    nc.sync.dma_start(t, cst_d[:, c0:c1])
                    _cst_cache[name] = t
                if dtype != F32:
                    tb = csts.tile([128, c1 - c0], dtype,
                                   name=f"cstb_{name}_{dtype}",
                                   tag=f"cstb_{name}_{dtype}")
                    nc.vector.tensor_copy(tb, t)
                    return tb
                return t

            S1 = cslab("S1", F32R)
            S2 = cslab("S2", F32R)
            Tmov = cslab("Tmov")
            IPb = cslab("IP_beta")
            IPu = cslab("IP_unit")
            IPub = cslab("IP_unit", BF16)
            IPexp = [cslab(f"IP_exp{j}") for j in range(r_exp + 1)]
            CL = {}
            for j in range(r_log + 1):
                for i in range(S_LOG):
                    if f"CL{j}_{i}" in slabs:
                        CL[(j, i)] = cslab(f"CL{j}_{i}", BF16)
            CE = {(j, i): cslab(f"CE{j}_{i}", BF16)
                  for j, blk in enumerate(bexp_blocks)
                  for i in range(1, len(blk))}

            def quad16(ps, stat_tile, mov_tile, first=True, last=True,
                       mov64=None):
                n = 0
                for h in (0, 64):
                    for j in range(8):
                        sl = slice(64 * j, 64 * j + 64)
                        mov = (mov64[h:h + 64, 0:64] if mov64 is not None
                               else mov_tile[h:h + 64, sl])
                        nc.tensor.matmul(
                            ps[h:h + 64, sl], stat_tile[h:h + 64, sl], mov,
                            start=first, stop=(last and n == 15),
                            tile_position=(h, h))
                        n += 1

            for g in range(ngrp):
                Xt = work.tile([128, 512], F32, name=f"X{g}", tag="X")
                Xt3 = Xt.rearrange("r (p c) -> r p c", p=8)
                nc.sync.dma_start(Xt3[0:64], x_r[g, 0])
                nc.sync.dma_start(Xt3[64:128], x_r[g, 1])

                # ---- congruence 1: A1 = X T^T ; W = alpha T A1 + beta I ----
                ps_a = psp.tile([128, 512], F32, name=f"psa{g}", tag="ps")
                quad16(ps_a, Xt, None, mov64=Tmov)
                A1 = work.tile([128, 512], F32R, name=f"A1_{g}", tag="A1")
                nc.scalar.activation(A1, ps_a, COPYF)
                ps_y = psp.tile([128, 512], F32, name=f"psy{g}", tag="ps")
                nc.tensor.matmul(ps_y, S1, A1, start=True, stop=True)
                Wt = work.tile([128, 512], BF16, name=f"W{g}", tag="W")
                nc.vector.scalar_tensor_tensor(Wt, ps_y, 1.0, IPb, MULT, ADD)

                # ---- log: Chebyshev chain T_k = 2 W T_{k-1} - T_{k-2} ----
                Tch = {1: Wt}
                for k in range(2, S_LOG + 1):
                    ps_t = psp.tile([128, 512], F32, name=f"pst{g}_{k}",
                                    tag="ps")
                    quad16(ps_t, Wt, Tch[k - 1])
                    Tk = work.tile([128, 512], BF16, name=f"T{g}_{k}",
                                   tag=f"T{k}")
                    sub_src = IPu if k == 2 else Tch[k - 2]
                    nc.vector.scalar_tensor_tensor(Tk, ps_t, 2.0, sub_src,
                                                   MULT, SUB)
                    Tch[k] = Tk
                Zl = Tch[S_LOG]

                # ---- log: doubled Clenshaw ----
                def emit_Q(ps_h, j, quad_mov=None):
                    first = True
                    for i in range(1, S_LOG):
                        if (j, i) in CL:
                            nc.tensor.matmul(ps_h, CL[(j, i)], Tch[i],
                                             start=first, stop=False)
                            first = False
                    if (j, 0) in CL:
                        nc.tensor.matmul(ps_h, CL[(j, 0)], IPub, start=first,
                                         stop=(quad_mov is None))
                        first = False
                    assert (not first) or quad_mov is not None
                    if quad_mov is not None:
                        quad16(ps_h, Zl, quad_mov, first=first, last=True)

                Bt = {}
                ps_h = psp.tile([128, 512], F32, name=f"psq{g}_{r_log}",
                                tag="ps")
                emit_Q(ps_h, r_log)
                Br = work.tile([128, 512], BF16, name=f"B{g}_{r_log}",
                               tag=f"B{r_log}")
                nc.scalar.activation(Br, ps_h, COPYF, scale=2.0)
                Bt[r_log] = Br
                for j in range(r_log - 1, 0, -1):
                    ps_h = psp.tile([128, 512], F32, name=f"psq{g}_{j}",
                                    tag="ps")
                    emit_Q(ps_h, j, quad_mov=Bt[j + 1])
                    Bj = work.tile([128, 512], BF16, name=f"B{g}_{j}",
                                   tag=f"B{j}")
                    if (j + 2) in Bt:
                        nc.vector.scalar_tensor_tensor(Bj, ps_h, 2.0,
                                                       Bt[j + 2], MULT, SUB)
                    else:
                        nc.scalar.activation(Bj, ps_h, COPYF, scale=2.0)
                    Bt[j] = Bj
                ps_h = psp.tile([128, 512], F32, name=f"psq{g}_0", tag="ps")
                emit_Q(ps_h, 0, quad_mov=Bt[1])
                Lt = work.tile([128, 512], F32, name=f"L{g}", tag="L")
                nc.vector.scalar_tensor_tensor(Lt, ps_h, 1.0, Bt[2],
                                               MULT, SUB)

                # ---- congruence 2: A2 = (2L) T^T ; V = T A2 / (2 zb) ----
                ps_b = psp.tile([128, 512], F32, name=f"psb{g}", tag="ps")
                quad16(ps_b, Lt, None, mov64=Tmov)
                A2 = work.tile([128, 512], F32R, name=f"A2_{g}", tag="A2")
                nc.scalar.activation(A2, ps_b, COPYF)
                ps_v = psp.tile([128, 512], F32, name=f"psv{g}", tag="ps")
                nc.tensor.matmul(ps_v, S2, A2, start=True, stop=True)
                Vt = work.tile([128, 512], BF16, name=f"V{g}", tag="V")
                nc.scalar.activation(Vt, ps_v, COPYF)

                # ---- exp: powers V^2, Ze = V^3 ----
                pe = {1: Vt}
                ps_2 = psp.tile([128, 512], F32, name=f"pv2{g}", tag="ps")
                quad16(ps_2, Vt, Vt)
                V2 = work.tile([128, 512], BF16, name=f"V2{g}", tag="V2")
                nc.scalar.activation(V2, ps_2, COPYF)
                pe[2] = V2
                ps_3 = psp.tile([128, 512], F32, name=f"pv3{g}", tag="ps")
                quad16(ps_3, Vt, V2)
                Ze = work.tile([128, 512], BF16, name=f"Ze{g}", tag="Ze")
                nc.scalar.activation(Ze, ps_3, COPYF)

                # ---- exp: Horner (monomial PS), cI first then quads ----
                He = None
                for j in range(r_exp, -1, -1):
                    blk = bexp_blocks[j]
                    ps_e = psp.tile([128, 512], F32, name=f"pse{g}_{j}",
                                    tag="ps")
                    first = True
                    for i in range(1, len(blk)):
                        nc.tensor.matmul(ps_e, CE[(j, i)], pe[i], start=first,
                                         stop=(He is None and
                                               i == len(blk) - 1))
                        first = False
                    assert He is not None or not first
                    if He is not None:
                        quad16(ps_e, Ze, He, first=first, last=True)
                    out_dt = F32 if j == 0 else BF16
                    name = f"O{g}" if j == 0 else f"He{g}_{j}"
                    Het = work.tile([128, 512], out_dt, name=name,
                                    tag=("O" if j == 0 else f"He{j}"))
                    nc.vector.scalar_tensor_tensor(Het, ps_e, 1.0, IPexp[j],
                                                   MULT, ADD)
                    He = Het

                He3 = He.rearrange("r (p c) -> r p c", p=8)
                nc.sync.dma_start(o_r[g, 0], He3[0:64])
                nc.sync.dma_start(o_r[g, 1], He3[64:128])

    nc.compile()
    return nc, blob


_CACHE = {}


def kernel(X, running_mean):
    key = running_mean.tobytes()[:256]
    if key not in _CACHE:
        hp = _host_prep(np.asarray(running_mean, dtype=np.float32))
        _CACHE[key] = _build_kernel(hp)
    nc, blob = _CACHE[key]

    X = np.ascontiguousarray(np.asarray(X, dtype=np.float32))
    in_maps = [{"x": X[i * SPC:(i + 1) * SPC], "cst": blob}
               for i in range(NCORES)]
    res = bass_utils.run_bass_kernel_spmd(
        nc, in_maps, core_ids=list(range(NCORES)),
        trace=bool(int(os.environ.get("K_TRACE", "0"))))
    out = np.concatenate([res.results[i]["out"] for i in range(NCORES)],
                         axis=0)
    kernel.last_exec_time_ns = res.exec_time_ns
    return out.astype(np.float32)


kernel.last_exec_time_ns = None


# revision 12
# speedup vs baseline: 1.1024x; 1.1024x over previous
"""BarycenterNorm (eval mode) Trainium2 kernel.

Math: the reference chain collapses to out_i = exp(T log(T X_i T^T) T^T)
with T = chol(B^-1).T (verified vs reference to 5e-9). With X and L
symmetric, both congruences avoid per-sample transposes on the PE
(out = lhsT.T @ rhs):
  A1 = X_i T^T    per-sample stationary X_i, shared moving T^T  (f32 quad)
  Y  = T A1       shared stationary diag(aT^T,aT^T), f32r       -> W
  W  = a*Y + b*I  spectrum in [-1,1]
  2L = cheb2-Clenshaw(W)    all-bf16, two-level Chebyshev basis:
       p(W) = sum_j Q_j(W) T_j(z), z = T_s(W); doubled recurrence
       Bt_j = 2(z Bt_{j+1} + Q_j) - Bt_{j+2} keeps the x2 out of Q.
  A2 = (2L) T^T   per-sample stationary, f32 quad
  V  = T A2/(2zb) shared f32r; spectrum [-1,1]
  out = p_exp(V)  monomial PS (coeffs O(1)), bf16

Layout: 16-sample groups; tiles [128,512]: samples 16g..16g+7 in
partitions 0-63, 16g+8..16g+15 in 64-127, each a [64,64] block along
free. Per-sample matmuls are 64x64 quadrant matmuls at (0,0)/(64,64).
Coefficient injection: K=128 (c*I) matmuls accumulate in PSUM; they must
be emitted BEFORE the quadrant matmuls of the same bank (HW quirk:
full-K accumulation after quadrant-mode start=True writes corrupts).
"""
import os
import sys

import numpy as np

sys.path.insert(0, "/opt/trn_rl_repo")

import concourse.bacc as bacc  # noqa: E402
import concourse.tile as tile  # noqa: E402
from concourse import mybir  # noqa: E402
from concourse import bass_utils  # noqa: E402

try:
    import axon_profile_shim  # noqa: F401
except Exception:
    pass

F32 = mybir.dt.float32
F32R = mybir.dt.float32r
BF16 = mybir.dt.bfloat16

C = 64
BATCH = 8192
NCORES = 8
SPC = BATCH // NCORES
NGRP = SPC // 16

N_LOG = int(os.environ.get("K_NLOG", 15))
S_LOG = int(os.environ.get("K_SLOG", 4))
N_EXP = int(os.environ.get("K_NEXP", 7))
S_EXP = int(os.environ.get("K_SEXP", 3))
A_MARGIN = 0.98
B_MARGIN = 6.5

MULT = mybir.AluOpType.mult
ADD = mybir.AluOpType.add
SUB = mybir.AluOpType.subtract
COPYF = mybir.ActivationFunctionType.Copy


def _cheb_coeffs(f, lo, hi, deg):
    k = np.arange(deg + 1)
    nw = np.cos((2 * k + 1) * np.pi / (2 * (deg + 1)))
    nx = 0.5 * (hi - lo) * nw + 0.5 * (lo + hi)
    return np.polynomial.chebyshev.chebfit(nw, f(nx), deg)


def _solve_two_level(a, s):
    """p = sum_{j,i} b[j,i] T_i(w) T_{js}(w); triangular solve."""
    n = len(a) - 1
    r = n // s
    rem = a.astype(np.float64).copy()
    b = np.zeros((r + 1, s))
    for j in range(r, -1, -1):
        for i in range(min(s - 1, n - j * s), 0, -1):
            m = j * s + i
            if j == 0:
                b[j, i] = rem[m]
                rem[m] = 0.0
            else:
                coef = 2.0 * rem[m]
                b[j, i] = coef
                rem[m] = 0.0
                rem[abs(j * s - i)] -= coef / 2.0
        b[j, 0] = rem[j * s]
        rem[j * s] = 0.0
    assert np.abs(rem).max() < 1e-10
    return b


def _host_prep(running_mean):
    B = running_mean[0].astype(np.float64)
    sev = np.linalg.eigvalsh(B)
    T = np.linalg.cholesky(np.linalg.inv(B)).T  # upper; T^T T = B^-1
    a = A_MARGIN / sev[-1]
    b = B_MARGIN / sev[0]
    alpha = 2.0 / (b - a)
    beta = -(a + b) / (b - a)
    zb = max(abs(np.log(a)), abs(np.log(b))) / sev[0]
    b_log = _solve_two_level(_cheb_coeffs(np.log, a, b, N_LOG), S_LOG)
    c_exp = np.polynomial.chebyshev.cheb2poly(
        _cheb_coeffs(lambda v: np.exp(zb * v), -1.0, 1.0, N_EXP))
    return dict(T=T, alpha=alpha, beta=beta, zb=zb, b_log=b_log,
                c_exp=c_exp, r_log=N_LOG // S_LOG, r_exp=N_EXP // S_EXP)


def _build_consts(hp):
    T = hp["T"].astype(np.float32)
    Tt = np.ascontiguousarray(T.T)
    alpha = np.float32(hp["alpha"])
    zb = np.float32(hp["zb"])
    I64 = np.eye(64, dtype=np.float32)

    slabs = {}
    cols = []

    def add(name, arr):
        c0 = sum(a.shape[1] for a in cols)
        cols.append(np.ascontiguousarray(arr, dtype=np.float32))
        slabs[name] = (c0, c0 + arr.shape[1])

    dTT = np.zeros((128, 128), np.float32)
    dTT[:64, :64] = Tt
    dTT[64:, 64:] = Tt
    add("S1", alpha * dTT)
    add("S2", dTT / (2.0 * zb))
    add("Tmov", np.concatenate([Tt, Tt], axis=0))  # [128, 64]

    ipat = np.zeros((128, 512), np.float32)
    for j in range(8):
        ipat[:64, 64 * j:64 * j + 64] = I64
        ipat[64:, 64 * j:64 * j + 64] = I64

    add("IP_beta", np.float32(hp["beta"]) * ipat)
    add("IP_unit", ipat)

    bexp_blocks = [hp["c_exp"][j * S_EXP:min(j * S_EXP + S_EXP, N_EXP + 1)]
                   for j in range(hp["r_exp"] + 1)]
    for j, blk in enumerate(bexp_blocks):
        add(f"IP_exp{j}", np.float32(blk[0]) * ipat)

    I128 = np.eye(128, dtype=np.float32)
    b_log = hp["b_log"]
    for j in range(hp["r_log"] + 1):
        for i in range(S_LOG):
            v = b_log[j, i] * (2.0 if j == 0 else 1.0)
            if v != 0.0:
                add(f"CL{j}_{i}", np.float32(v) * I128)
    for j, blk in enumerate(bexp_blocks):
        for i in range(1, len(blk)):
            add(f"CE{j}_{i}", np.float32(blk[i]) * I128)

    return np.concatenate(cols, axis=1), slabs, bexp_blocks


def _build_kernel(hp, ngrp=None):
    if ngrp is None:
        ngrp = int(os.environ.get("K_NGRP", NGRP))
    blob, slabs, bexp_blocks = _build_consts(hp)
    r_log, r_exp = hp["r_log"], hp["r_exp"]

    nc = bacc.Bacc("TRN2", target_bir_lowering=False, debug=False)
    x_d = nc.dram_tensor("x", [SPC, C, C], F32, kind="ExternalInput").ap()
    cst_d = nc.dram_tensor("cst", [128, blob.shape[1]], F32,
                           kind="ExternalInput").ap()
    out_d = nc.dram_tensor("out", [SPC, C, C], F32, kind="ExternalOutput").ap()

    x_r = x_d.rearrange("(g two p) r c -> g two r p c", g=NGRP, two=2)
    o_r = out_d.rearrange("(g two p) r c -> g two r p c", g=NGRP, two=2)

    with tile.TileContext(nc) as tc:
        with tc.tile_pool(name="csts", bufs=1) as csts, \
             tc.tile_pool(name="work", bufs=3) as work, \
             tc.tile_pool(name="psp", bufs=7, space="PSUM") as psp:

            _cst_cache = {}

            def cslab(name, dtype=F32):
                c0, c1 = slabs[name]
                if name in _cst_cache:
                    t = _cst_cache[name]
                else:
                    t = csts.tile([128, c1 - c0], F32, name=f"cst_{name}",
                                  tag=f"cst_{name}")
                    nc.sync.dma_start(t, cst_d[:, c0:c1])
                    _cst_cache[name] = t
                if dtype != F32:
                    tb = csts.tile([128, c1 - c0], dtype,
                                   name=f"cstb_{name}_{dtype}",
                                   tag=f"cstb_{name}_{dtype}")
                    nc.vector.tensor_copy(tb, t)
                    return tb
                return t

            S1 = cslab("S1", F32R)
            S2 = cslab("S2", F32R)
            Tmov = cslab("Tmov")
            IPb = cslab("IP_beta")
            IPu = cslab("IP_unit")
            IPub = cslab("IP_unit", BF16)
            IPexp = [cslab(f"IP_exp{j}") for j in range(r_exp + 1)]
            CL = {}
            for j in range(r_log + 1):
                for i in range(S_LOG):
                    if f"CL{j}_{i}" in slabs:
                        CL[(j, i)] = cslab(f"CL{j}_{i}", BF16)
            CE = {(j, i): cslab(f"CE{j}_{i}", BF16)
                  for j, blk in enumerate(bexp_blocks)
                  for i in range(1, len(blk))}

            def quad16(ps, stat_tile, mov_tile, first=True, last=True,
                       mov64=None):
                # interleave row-halves so each LDWEIGHTS overlaps the
                # other half's in-flight matmul (different row_grp)
                n = 0
                for j in range(8):
                    for h in (0, 64):
                        sl = slice(64 * j, 64 * j + 64)
                        mov = (mov64[h:h + 64, 0:64] if mov64 is not None
                               else mov_tile[h:h + 64, sl])
                        nc.tensor.matmul(
                            ps[h:h + 64, sl], stat_tile[h:h + 64, sl], mov,
                            start=first, stop=(last and n == 15),
                            tile_position=(h, h))
                        n += 1

            for g in range(ngrp):
                Xt = work.tile([128, 512], F32, name=f"X{g}", tag="X")
                Xt3 = Xt.rearrange("r (p c) -> r p c", p=8)
                nc.sync.dma_start(Xt3[0:64], x_r[g, 0])
                nc.sync.dma_start(Xt3[64:128], x_r[g, 1])

                # ---- congruence 1: A1 = X T^T ; W = alpha T A1 + beta I ----
                ps_a = psp.tile([128, 512], F32, name=f"psa{g}", tag="ps")
                quad16(ps_a, Xt, None, mov64=Tmov)
                A1 = work.tile([128, 512], F32R, name=f"A1_{g}", tag="A1")
                nc.scalar.activation(A1, ps_a, COPYF)
                ps_y = psp.tile([128, 512], F32, name=f"psy{g}", tag="ps")
                nc.tensor.matmul(ps_y, S1, A1, start=True, stop=True)
                Wt = work.tile([128, 512], BF16, name=f"W{g}", tag="W")
                nc.vector.scalar_tensor_tensor(Wt, ps_y, 1.0, IPb, MULT, ADD)

                # ---- log: Chebyshev chain T_k = 2 W T_{k-1} - T_{k-2} ----
                Tch = {1: Wt}
                for k in range(2, S_LOG + 1):
                    ps_t = psp.tile([128, 512], F32, name=f"pst{g}_{k}",
                                    tag="ps")
                    quad16(ps_t, Wt, Tch[k - 1])
                    Tk = work.tile([128, 512], BF16, name=f"T{g}_{k}",
                                   tag=f"T{k}")
                    sub_src = IPu if k == 2 else Tch[k - 2]
                    nc.vector.scalar_tensor_tensor(Tk, ps_t, 2.0, sub_src,
                                                   MULT, SUB)
                    Tch[k] = Tk
                Zl = Tch[S_LOG]

                # ---- log: doubled Clenshaw ----
                def emit_Q(ps_h, j, quad_mov=None):
                    first = True
                    for i in range(1, S_LOG):
                        if (j, i) in CL:
                            nc.tensor.matmul(ps_h, CL[(j, i)], Tch[i],
                                             start=first, stop=False)
                            first = False
                    if (j, 0) in CL:
                        nc.tensor.matmul(ps_h, CL[(j, 0)], IPub, start=first,
                                         stop=(quad_mov is None))
                        first = False
                    assert (not first) or quad_mov is not None
                    if quad_mov is not None:
                        quad16(ps_h, Zl, quad_mov, first=first, last=True)

                Bt = {}
                ps_h = psp.tile([128, 512], F32, name=f"psq{g}_{r_log}",
                                tag="ps")
                emit_Q(ps_h, r_log)
                Br = work.tile([128, 512], BF16, name=f"B{g}_{r_log}",
                               tag=f"B{r_log}")
                nc.scalar.activation(Br, ps_h, COPYF, scale=2.0)
                Bt[r_log] = Br
                for j in range(r_log - 1, 0, -1):
                    ps_h = psp.tile([128, 512], F32, name=f"psq{g}_{j}",
                                    tag="ps")
                    emit_Q(ps_h, j, quad_mov=Bt[j + 1])
                    Bj = work.tile([128, 512], BF16, name=f"B{g}_{j}",
                                   tag=f"B{j}")
                    if (j + 2) in Bt:
                        nc.vector.scalar_tensor_tensor(Bj, ps_h, 2.0,
                                                       Bt[j + 2], MULT, SUB)
                    else:
                        nc.scalar.activation(Bj, ps_h, COPYF, scale=2.0)
                    Bt[j] = Bj
                ps_h = psp.tile([128, 512], F32, name=f"psq{g}_0", tag="ps")
                emit_Q(ps_h, 0, quad_mov=Bt[1])
                Lt = work.tile([128, 512], F32, name=f"L{g}", tag="L")
                nc.vector.scalar_tensor_tensor(Lt, ps_h, 1.0, Bt[2],
                                               MULT, SUB)

                # ---- congruence 2: A2 = (2L) T^T ; V = T A2 / (2 zb) ----
                ps_b = psp.tile([128, 512], F32, name=f"psb{g}", tag="ps")
                quad16(ps_b, Lt, None, mov64=Tmov)
                A2 = work.tile([128, 512], F32R, name=f"A2_{g}", tag="A2")
                nc.scalar.activation(A2, ps_b, COPYF)
                ps_v = psp.tile([128, 512], F32, name=f"psv{g}", tag="ps")
                nc.tensor.matmul(ps_v, S2, A2, start=True, stop=True)
                Vt = work.tile([128, 512], BF16, name=f"V{g}", tag="V")
                nc.scalar.activation(Vt, ps_v, COPYF)

                # ---- exp: powers V^2, Ze = V^3 ----
                pe = {1: Vt}
                ps_2 = psp.tile([128, 512], F32, name=f"pv2{g}", tag="ps")
                quad16(ps_2, Vt, Vt)
                V2 = work.tile([128, 512], BF16, name=f"V2{g}", tag="V2")
                nc.scalar.activation(V2, ps_2, COPYF)
                pe[2] = V2
                ps_3 = psp.tile([128, 512], F32, name=f"pv3{g}", tag="ps")
                quad16(ps_3, Vt, V2)
                Ze = work.tile([128, 512], BF16, name=f"Ze{g}", tag="Ze")
                nc.scalar.activation(Ze, ps_3, COPYF)

                # ---- exp: Horner (monomial PS), cI first then quads ----
                He = None
                for j in range(r_exp, -1, -1):
                    blk = bexp_blocks[j]
                    ps_e = psp.tile([128, 512], F32, name=f"pse{g}_{j}",
                                    tag="ps")
                    first = True
                    for i in range(1, len(blk)):
                        nc.tensor.matmul(ps_e, CE[(j, i)], pe[i], start=first,
                                         stop=(He is None and
                                               i == len(blk) - 1))
                        first = False
                    assert He is not None or not first
                    if He is not None:
                        quad16(ps_e, Ze, He, first=first, last=True)
                    out_dt = F32 if j == 0 else BF16
                    name = f"O{g}" if j == 0 else f"He{g}_{j}"
                    Het = work.tile([128, 512], out_dt, name=name,
                                    tag=("O" if j == 0 else f"He{j}"))
                    nc.vector.scalar_tensor_tensor(Het, ps_e, 1.0, IPexp[j],
                                                   MULT, ADD)
                    He = Het

                He3 = He.rearrange("r (p c) -> r p c", p=8)
                nc.sync.dma_start(o_r[g, 0], He3[0:64])
                nc.sync.dma_start(o_r[g, 1], He3[64:128])

    nc.compile()
    return nc, blob


_CACHE = {}


def kernel(X, running_mean):
    key = running_mean.tobytes()[:256]
    if key not in _CACHE:
        hp = _host_prep(np.asarray(running_mean, dtype=np.float32))
        _CACHE[key] = _build_kernel(hp)
    nc, blob = _CACHE[key]

    X = np.ascontiguousarray(np.asarray(X, dtype=np.float32))
    in_maps = [{"x": X[i * SPC:(i + 1) * SPC], "cst": blob}
               for i in range(NCORES)]
    res = bass_utils.run_bass_kernel_spmd(
        nc, in_maps, core_ids=list(range(NCORES)),
        trace=bool(int(os.environ.get("K_TRACE", "0"))))
    out = np.concatenate([res.results[i]["out"] for i in range(NCORES)],
                         axis=0)
    kernel.last_exec_time_ns = res.exec_time_ns
    return out.astype(np.float32)


kernel.last_exec_time_ns = None
